# revision 31
# baseline (speedup 1.0000x reference)
"""Trainium2 Bass kernel for causal multi-head attention.

Problem: B=4, T=2048, D=1024, H=16, HD=64, fp32, causal, scale=1/sqrt(D).

Sharding: 4-way batch x 2-way head-group over 8 cores. Core c=(b,g) computes
heads g*8..g*8+7 for batch b and returns the partial output projection
(contracted over its 512 context columns); the host sums the two partials
per batch element and adds bo.

Per-core dataflow (all matmuls in float32r, which runs at full PE rate for
moving free-dim >= 256):
  - Host passes q[b].T etc., so no on-device transposes are needed.
  - Q^T,K^T projections produce [feat_part, token] layouts directly
    (lhsT = W slice, rhs = x^T tile); the 1/sqrt(D) scale and biases are
    folded into the PSUM->SBUF copyback on the vector engine.
  - V projection produces natural [token_part, feat] layout (lhsT = x^T
    slice, rhs = Wv). V is stored with a ones-column appended per head so
    the PV matmul also yields the softmax denominator.
  - Scores are computed transposed, S^T[tk_part, tq_free] (lhsT = K^T
    block, rhs = Q^T tile). Causal masking adds -1e30 via one extra
    matmul (lhsT = identity, rhs = precomputed mask, both bf16) on
    diagonal blocks; blocks above the diagonal are skipped entirely.
  - exp on the scalar engine (scores are O(1) so no max-subtraction is
    needed), then PV accumulates ctx^T[dv, tq] + denominator row.
  - ctx is normalized with a reciprocal + DRAM-bounce partition-broadcast
    DMA + multiply into a resident ctx^T buffer, which feeds the output
    projection directly as lhsT.
"""

import numpy as np
from contextlib import ExitStack

import ml_dtypes
import concourse.bass as bass
import concourse.tile as tile
from concourse import bacc
from concourse import mybir
from concourse.bass_utils import run_bass_kernel_spmd

F32 = mybir.dt.float32
F32R = mybir.dt.float32r
BF16 = mybir.dt.bfloat16
AF = mybir.ActivationFunctionType
OP = mybir.AluOpType


def build_mha_core(T, D, F, DOUT, HD=64, TQ=512, scale=1.0, num_devices=1):
    """Build the per-core Bass program.

    T: tokens, D: model dim, F: feature columns owned by this core,
    DOUT: output projection width, HD: head dim, TQ: tq tile width.
    """
    NH = F // HD        # local heads
    DT = D // 128       # contraction tiles for projections
    FT = F // 128       # feature 128-tiles
    NTOK = T // 128     # token 128-tiles
    NTQ = T // TQ       # tq tiles
    NR = TQ // 128      # 128-blocks per tq tile
    NCH = min(512, DOUT)
    NO = DOUT // NCH
    HPF = 128 // HD     # heads per feature tile

    nc = bacc.Bacc(None, target_bir_lowering=False, debug=False, num_devices=num_devices)

    qT = nc.dram_tensor("qT", [D, T], F32, kind="ExternalInput")
    kTd = nc.dram_tensor("kT", [D, T], F32, kind="ExternalInput")
    vTd = nc.dram_tensor("vT", [D, T], F32, kind="ExternalInput")
    Wq = nc.dram_tensor("Wq", [D, F], F32, kind="ExternalInput")
    Wk = nc.dram_tensor("Wk", [D, F], F32, kind="ExternalInput")
    Wv = nc.dram_tensor("Wv", [D, F], F32, kind="ExternalInput")
    Wo = nc.dram_tensor("Wo", [F, DOUT], F32, kind="ExternalInput")
    bq = nc.dram_tensor("bq", [128, FT], F32, kind="ExternalInput")
    bk = nc.dram_tensor("bk", [128, FT], F32, kind="ExternalInput")
    bv = nc.dram_tensor("bv", [1, F], F32, kind="ExternalInput")
    ones = nc.dram_tensor("ones", [1, 1], F32, kind="ExternalInput")
    mneg = nc.dram_tensor("mneg", [128, NR, TQ], BF16, kind="ExternalInput")
    ident = nc.dram_tensor("ident", [128, 128], BF16, kind="ExternalInput")
    out = nc.dram_tensor("out", [T, DOUT], F32, kind="ExternalOutput")

    with tile.TileContext(nc) as tc:
        with ExitStack() as ctx:
            persist = ctx.enter_context(tc.tile_pool(name="persist", bufs=1))
            QT_sb = persist.tile([128, FT, T], BF16)
            KT_sb = persist.tile([128, FT, T], BF16)
            VA_sb = persist.tile([128, NTOK, NH, HD + 1], F32R)
            CTX_sb = persist.tile([128, FT, T], F32R)
            bq_sb = persist.tile([128, FT], F32)
            bk_sb = persist.tile([128, FT], F32)
            bv_sb = persist.tile([128, F], F32)
            mneg_sb = persist.tile([128, NR, TQ], BF16)
            ident_sb = persist.tile([128, 128], BF16)

            # psum pools are shared across all phases (no pool barriers);
            # pmain(2) + pS(3) + pO(3) = 8 banks exactly.
            ppool = ctx.enter_context(tc.tile_pool(name="pmain", bufs=2, space="PSUM"))
            pS = ctx.enter_context(tc.tile_pool(name="pS", bufs=3, space="PSUM"))
            pO = ctx.enter_context(tc.tile_pool(name="pO", bufs=3, space="PSUM"))
            ptile = ctx.enter_context(tc.tile_pool(name="ptile", bufs=3))

            # ---- Phase 1+2: projections interleaved with attention (tj-major) ----
            with tc.tile_pool(name="wqkv", bufs=3) as wpool, \
                 tc.tile_pool(name="xin", bufs=DT + 5) as xpool, \
                 tc.tile_pool(name="den", bufs=3) as denp, \
                 tc.tile_pool(name="dend", bufs=3, space="DRAM") as dendp:
                Wv_sb = wpool.tile([128, DT, F], F32R, tag="w")
                Wq_sb = wpool.tile([128, DT, F], F32R, tag="w")
                Wk_sb = wpool.tile([128, DT, F], F32R, tag="w")

                def load_w(dst, wdram):
                    wr = wdram[:].rearrange("(dt p) f -> p dt f", p=128).bitcast(F32R)
                    for dt in range(DT):
                        nc.sync.dma_start(dst[:, dt:dt + 1, :], wr[:, dt:dt + 1, :])

                def load_x(xdram, tj):
                    ts = []
                    for dt in range(DT):
                        t_ = xpool.tile([128, TQ], F32R, tag="xin")
                        nc.sync.dma_start(
                            t_[:], xdram[dt * 128:(dt + 1) * 128, tj * TQ:(tj + 1) * TQ].bitcast(F32R))
                        ts.append(t_)
                    return ts

                load_w(Wv_sb, Wv)
                nc.sync.dma_start(bv_sb[:], bv[:].to_broadcast([128, F]))

                def v_proj(tj):
                    vt = load_x(vTd, tj)
                    for c in range(NR):
                        tt = tj * NR + c
                        ps = ppool.tile([128, TQ], F32, tag="pproj")
                        psv = ps[:, :F]
                        for dt in range(DT):
                            nc.tensor.matmul(
                                psv,
                                lhsT=vt[dt][:, c * 128:(c + 1) * 128],
                                rhs=Wv_sb[:, dt, :],
                                start=(dt == 0), stop=(dt == DT - 1))
                        for h in range(NH):
                            nc.vector.tensor_tensor(
                                VA_sb[:, tt, h, 0:HD],
                                psv[:, h * HD:(h + 1) * HD],
                                bv_sb[:, h * HD:(h + 1) * HD],
                                OP.add)

                def qk_proj(which, tj):
                    dst, Wsb, xdram, bsb, sc = which
                    xt = load_x(xdram, tj)
                    for ft in range(FT):
                        ps = ppool.tile([128, TQ], F32, tag="pproj")
                        for dt in range(DT):
                            nc.tensor.matmul(
                                ps[:],
                                lhsT=Wsb[:, dt, ft * 128:(ft + 1) * 128],
                                rhs=xt[dt][:],
                                start=(dt == 0), stop=(dt == DT - 1))
                        nc.vector.tensor_scalar(
                            dst[:, ft, tj * TQ:(tj + 1) * TQ], ps[:],
                            sc, bsb[:, ft:ft + 1], OP.mult, OP.add)

                QSPEC = (QT_sb, Wq_sb, qT, bq_sb, scale)
                KSPEC = (KT_sb, Wk_sb, kTd, bk_sb, 1.0)

                def attention(h, tj):
                    ft, po = h // HPF, (h % HPF) * HD
                    QhT = QT_sb[po:po + HD, ft, :]
                    KhT = KT_sb[po:po + HD, ft, :]
                    nblk = NR * tj + NR

                    def emit_S(i):
                        ps = pS.tile([128, TQ], F32, tag="pS")
                        r = i - NR * tj
                        nc.tensor.matmul(
                            ps[:],
                            lhsT=KhT[:, i * 128:(i + 1) * 128],
                            rhs=QhT[:, tj * TQ:(tj + 1) * TQ],
                            start=True, stop=(r < 0))
                        if r >= 0:
                            nc.tensor.matmul(
                                ps[:],
                                lhsT=ident_sb[:],
                                rhs=mneg_sb[:, r, :],
                                start=False, stop=True)
                        return ps

                    po_t = pO.tile([HD + 1, TQ], F32, tag="pO")
                    ps_cur = emit_S(0)
                    for i in range(nblk):
                        ps_next = emit_S(i + 1) if i + 1 < nblk else None
                        pt = ptile.tile([128, TQ], F32R, tag="pt")
                        nc.scalar.activation(pt[:], ps_cur[:], AF.Exp)
                        nc.tensor.matmul(
                            po_t[:],
                            lhsT=VA_sb[:, i, h, :],
                            rhs=pt[:],
                            start=(i == 0), stop=(i == nblk - 1))
                        ps_cur = ps_next
                    den1 = denp.tile([1, TQ], F32, tag="den1")
                    nc.vector.reciprocal(den1[:], po_t[HD:HD + 1, :])
                    dend = dendp.tile([1, TQ], F32, tag="dend")
                    nc.sync.dma_start(dend[:], den1[:])
                    denr = denp.tile([HD, TQ], F32, tag="denr")
                    nc.sync.dma_start(denr[:], dend[0:1, :].to_broadcast([HD, TQ]))
                    nc.vector.tensor_tensor(
                        CTX_sb[po:po + HD, ft, tj * TQ:(tj + 1) * TQ],
                        po_t[0:HD, :], denr[:], OP.mult)

                def qk_proj_ft(which, tj, xt, ft):
                    dst, Wsb, xdram, bsb, sc = which
                    ps = ppool.tile([128, TQ], F32, tag="pproj")
                    for dt in range(DT):
                        nc.tensor.matmul(
                            ps[:],
                            lhsT=Wsb[:, dt, ft * 128:(ft + 1) * 128],
                            rhs=xt[dt][:],
                            start=(dt == 0), stop=(dt == DT - 1))
                    nc.vector.tensor_scalar(
                        dst[:, ft, tj * TQ:(tj + 1) * TQ], ps[:],
                        sc, bsb[:, ft:ft + 1], OP.mult, OP.add)

                # prologue: V for tj=0, then per-feature-tile Q/K proj
                # interleaved with that tile's two heads of attention, so the
                # scalar engine starts exp as early as possible.
                v_proj(0)
                load_w(Wq_sb, Wq)
                nc.sync.dma_start(bq_sb[:], bq[:])
                nc.sync.dma_start(mneg_sb[:], mneg[:])
                nc.sync.dma_start(ident_sb[:], ident[:])
                qk_proj(QSPEC, 0)
                load_w(Wk_sb, Wk)
                nc.sync.dma_start(bk_sb[:], bk[:])
                nc.sync.dma_start(
                    VA_sb[:].rearrange("p a b c -> p (a b) c")[:, :, HD:HD + 1],
                    ones[0:1, 0:1].to_broadcast([128, NTOK * NH, 1]).bitcast(F32R))
                qk_proj(KSPEC, 0)

                for tj in range(NTQ):
                    for h in range(NH):
                        attention(h, tj)
                        if tj + 1 < NTQ:
                            if h == 1:
                                v_proj(tj + 1)
                            elif h == 3:
                                qk_proj(QSPEC, tj + 1)
                            elif h == 5:
                                qk_proj(KSPEC, tj + 1)

            # ---- Phase 3: output projection ----
            with tc.tile_pool(name="wom", bufs=1) as wop, \
                 tc.tile_pool(name="osb", bufs=3) as osb:
                Wo_sb = wop.tile([128, FT, DOUT], F32R)
                nc.sync.dma_start(
                    Wo_sb[:], Wo[:].rearrange("(ft p) n -> p ft n", p=128).bitcast(F32R))
                for tt in range(NTOK):
                    for n in range(NO):
                        ps = ppool.tile([128, NCH], F32, tag="pproj")
                        for ft in range(FT):
                            nc.tensor.matmul(
                                ps[:],
                                lhsT=CTX_sb[:, ft, tt * 128:(tt + 1) * 128],
                                rhs=Wo_sb[:, ft, n * NCH:(n + 1) * NCH],
                                start=(ft == 0), stop=(ft == FT - 1))
                        ot = osb.tile([128, NCH], F32, tag="ot")
                        nc.vector.tensor_copy(ot[:], ps[:])
                        nc.sync.dma_start(
                            out[tt * 128:(tt + 1) * 128, n * NCH:(n + 1) * NCH], ot[:])

    nc.compile()
    return nc


def make_mask(TQ=512, NR=4):
    """mneg[p, r, f] = -1e30 where tk > tq (tk = 128*i+p, tq = tj*TQ+f, r = i-NR*tj)."""
    p = np.arange(128)[:, None, None]
    r = np.arange(NR)[None, :, None]
    f = np.arange(TQ)[None, None, :]
    m = np.where(f < p + 128 * r, np.float32(-1e30), np.float32(0.0))
    return m.astype(ml_dtypes.bfloat16)


def make_core_inputs(q_b, k_b, v_b, Wq, bq, Wk, bk, Wv, bv, Wo, fsl, scale, TQ=512):
    """Build the in_map for one core. fsl = feature slice for this core's heads."""
    F = fsl.stop - fsl.start
    FT = F // 128
    NR = TQ // 128
    return {
        "qT": np.ascontiguousarray(q_b.T),
        "kT": np.ascontiguousarray(k_b.T),
        "vT": np.ascontiguousarray(v_b.T),
        "Wq": np.ascontiguousarray(Wq[:, fsl]),
        "Wk": np.ascontiguousarray(Wk[:, fsl]),
        "Wv": np.ascontiguousarray(Wv[:, fsl]),
        "Wo": np.ascontiguousarray(Wo[fsl, :]),
        "bq": np.ascontiguousarray((bq[fsl] * scale).reshape(FT, 128).T),
        "bk": np.ascontiguousarray(bk[fsl].reshape(FT, 128).T),
        "bv": np.ascontiguousarray(bv[fsl].reshape(1, F)),
        "ones": np.ones((1, 1), np.float32),
        "mneg": make_mask(TQ, NR),
        "ident": np.eye(128, dtype=np.float32).astype(ml_dtypes.bfloat16),
    }


_CACHE = {}


def kernel(q, k, v, Wq, bq, Wk, bk, Wv, bv, Wo, bo, _trace=False):
    B, T, D = q.shape
    H, HD = 16, 64
    scale = np.float32(1.0 / np.sqrt(D))
    n_cores = 8
    gpb = n_cores // B            # head-groups per batch element (2)
    F = D // gpb                  # feature columns per core (512)

    key = (T, D, F)
    if key not in _CACHE:
        _CACHE[key] = build_mha_core(T=T, D=D, F=F, DOUT=D, HD=HD, TQ=512,
                                     scale=float(scale), num_devices=n_cores)
    nc = _CACHE[key]

    q = np.asarray(q, np.float32)
    k = np.asarray(k, np.float32)
    v = np.asarray(v, np.float32)
    in_maps = []
    for c in range(n_cores):
        b, g = c // gpb, c % gpb
        fsl = slice(g * F, (g + 1) * F)
        in_maps.append(make_core_inputs(
            q[b], k[b], v[b], Wq, bq, Wk, bk, Wv, bv, Wo, fsl, scale))

    res = run_bass_kernel_spmd(nc, in_maps, list(range(n_cores)), trace=_trace)
    out = np.zeros((B, T, D), np.float32)
    for c in range(n_cores):
        out[c // gpb] += res.results[c]["out"]
    out += np.asarray(bo, np.float32)
    if _trace:
        kernel.last_exec_time_ns = res.exec_time_ns
    return out


# revision 34
# speedup vs baseline: 1.0806x; 1.0806x over previous
"""Trainium2 Bass kernel for causal multi-head attention.

Problem: B=4, T=2048, D=1024, H=16, HD=64, fp32, causal, scale=1/sqrt(D).

Sharding: 4-way batch x 2-way head-group over 8 cores. Core c=(b,g) computes
heads g*8..g*8+7 for batch b and returns the partial output projection
(contracted over its 512 context columns); the host sums the two partials
per batch element and adds bo.

Per-core dataflow (all matmuls in float32r, which runs at full PE rate for
moving free-dim >= 256):
  - Host passes q[b].T etc., so no on-device transposes are needed.
  - Q^T,K^T projections produce [feat_part, token] layouts directly
    (lhsT = W slice, rhs = x^T tile); the 1/sqrt(D) scale and biases are
    folded into the PSUM->SBUF copyback on the vector engine.
  - V projection produces natural [token_part, feat] layout (lhsT = x^T
    slice, rhs = Wv). V is stored with a ones-column appended per head so
    the PV matmul also yields the softmax denominator.
  - Scores are computed transposed, S^T[tk_part, tq_free] (lhsT = K^T
    block, rhs = Q^T tile). Causal masking adds -1e30 via one extra
    matmul (lhsT = identity, rhs = precomputed mask, both bf16) on
    diagonal blocks; blocks above the diagonal are skipped entirely.
  - exp on the scalar engine (scores are O(1) so no max-subtraction is
    needed), then PV accumulates ctx^T[dv, tq] + denominator row.
  - ctx is normalized with a reciprocal + DRAM-bounce partition-broadcast
    DMA + multiply into a resident ctx^T buffer, which feeds the output
    projection directly as lhsT.
"""

import numpy as np
from contextlib import ExitStack

import ml_dtypes
import concourse.bass as bass
import concourse.tile as tile
from concourse import bacc
from concourse import mybir
from concourse.bass_utils import run_bass_kernel_spmd

F32 = mybir.dt.float32
F32R = mybir.dt.float32r
BF16 = mybir.dt.bfloat16
AF = mybir.ActivationFunctionType
OP = mybir.AluOpType


def build_mha_core(T, D, F, DOUT, HD=64, TQ=512, scale=1.0, num_devices=1):
    """Build the per-core Bass program.

    T: tokens, D: model dim, F: feature columns owned by this core,
    DOUT: output projection width, HD: head dim, TQ: tq tile width.
    """
    NH = F // HD        # local heads
    DT = D // 128       # contraction tiles for projections
    FT = F // 128       # feature 128-tiles
    NTOK = T // 128     # token 128-tiles
    NTQ = T // TQ       # tq tiles
    NR = TQ // 128      # 128-blocks per tq tile
    NCH = min(512, DOUT)
    NO = DOUT // NCH
    HPF = 128 // HD     # heads per feature tile

    nc = bacc.Bacc(None, target_bir_lowering=False, debug=False, num_devices=num_devices)

    qT = nc.dram_tensor("qT", [D, T], F32, kind="ExternalInput")
    kTd = nc.dram_tensor("kT", [D, T], F32, kind="ExternalInput")
    vTd = nc.dram_tensor("vT", [D, T], F32, kind="ExternalInput")
    Wq = nc.dram_tensor("Wq", [D, F], F32, kind="ExternalInput")
    Wk = nc.dram_tensor("Wk", [D, F], F32, kind="ExternalInput")
    Wv = nc.dram_tensor("Wv", [D, F], F32, kind="ExternalInput")
    Wo = nc.dram_tensor("Wo", [F, DOUT], F32, kind="ExternalInput")
    bq = nc.dram_tensor("bq", [128, FT], F32, kind="ExternalInput")
    bk = nc.dram_tensor("bk", [128, FT], F32, kind="ExternalInput")
    bv = nc.dram_tensor("bv", [1, F], F32, kind="ExternalInput")
    ones = nc.dram_tensor("ones", [1, 1], F32, kind="ExternalInput")
    mneg = nc.dram_tensor("mneg", [128, NR, TQ], BF16, kind="ExternalInput")
    ident = nc.dram_tensor("ident", [128, 128], BF16, kind="ExternalInput")
    out = nc.dram_tensor("out", [T, DOUT], F32, kind="ExternalOutput")

    with tile.TileContext(nc) as tc:
        with ExitStack() as ctx:
            persist = ctx.enter_context(tc.tile_pool(name="persist", bufs=1))
            QT_sb = persist.tile([128, FT, T], BF16)
            KT_sb = persist.tile([128, FT, T], BF16)
            VA_sb = persist.tile([128, NTOK, NH, HD + 1], F32R)
            CTX_sb = persist.tile([128, FT, T], F32R)
            bq_sb = persist.tile([128, FT], F32)
            bk_sb = persist.tile([128, FT], F32)
            bv_sb = persist.tile([128, F], F32)
            mneg_sb = persist.tile([128, NR, TQ], BF16)
            ident_sb = persist.tile([128, 128], BF16)

            # psum pools are shared across all phases (no pool barriers);
            # pmain(2) + pS(3) + pO(3) = 8 banks exactly.
            ppool = ctx.enter_context(tc.tile_pool(name="pmain", bufs=2, space="PSUM"))
            pS = ctx.enter_context(tc.tile_pool(name="pS", bufs=3, space="PSUM"))
            pO = ctx.enter_context(tc.tile_pool(name="pO", bufs=3, space="PSUM"))
            ptile = ctx.enter_context(tc.tile_pool(name="ptile", bufs=3))

            # ---- Phase 1+2: projections interleaved with attention (tj-major) ----
            with tc.tile_pool(name="wqkv", bufs=3) as wpool, \
                 tc.tile_pool(name="xin", bufs=DT + 5) as xpool, \
                 tc.tile_pool(name="den", bufs=3) as denp, \
                 tc.tile_pool(name="dend", bufs=3, space="DRAM") as dendp:
                Wv_sb = wpool.tile([128, DT, F], F32R, tag="w")
                Wq_sb = wpool.tile([128, DT, F], F32R, tag="w")
                Wk_sb = wpool.tile([128, DT, F], F32R, tag="w")

                def load_w(dst, wdram):
                    wr = wdram[:].rearrange("(dt p) f -> p dt f", p=128).bitcast(F32R)
                    for dt in range(DT):
                        nc.sync.dma_start(dst[:, dt:dt + 1, :], wr[:, dt:dt + 1, :])

                def load_x(xdram, tj):
                    ts = []
                    for dt in range(DT):
                        t_ = xpool.tile([128, TQ], F32R, tag="xin")
                        nc.sync.dma_start(
                            t_[:], xdram[dt * 128:(dt + 1) * 128, tj * TQ:(tj + 1) * TQ].bitcast(F32R))
                        ts.append(t_)
                    return ts

                load_w(Wv_sb, Wv)
                nc.sync.dma_start(bv_sb[:], bv[:].to_broadcast([128, F]))

                def v_proj(tj):
                    vt = load_x(vTd, tj)
                    for c in range(NR):
                        tt = tj * NR + c
                        ps = ppool.tile([128, TQ], F32, tag="pproj")
                        psv = ps[:, :F]
                        for dt in range(DT):
                            nc.tensor.matmul(
                                psv,
                                lhsT=vt[dt][:, c * 128:(c + 1) * 128],
                                rhs=Wv_sb[:, dt, :],
                                start=(dt == 0), stop=(dt == DT - 1))
                        for h in range(NH):
                            nc.vector.tensor_tensor(
                                VA_sb[:, tt, h, 0:HD],
                                psv[:, h * HD:(h + 1) * HD],
                                bv_sb[:, h * HD:(h + 1) * HD],
                                OP.add)

                def qk_proj(which, tj):
                    dst, Wsb, xdram, bsb, sc = which
                    xt = load_x(xdram, tj)
                    for ft in range(FT):
                        ps = ppool.tile([128, TQ], F32, tag="pproj")
                        for dt in range(DT):
                            nc.tensor.matmul(
                                ps[:],
                                lhsT=Wsb[:, dt, ft * 128:(ft + 1) * 128],
                                rhs=xt[dt][:],
                                start=(dt == 0), stop=(dt == DT - 1))
                        nc.vector.tensor_scalar(
                            dst[:, ft, tj * TQ:(tj + 1) * TQ], ps[:],
                            sc, bsb[:, ft:ft + 1], OP.mult, OP.add)

                QSPEC = (QT_sb, Wq_sb, qT, bq_sb, scale)
                KSPEC = (KT_sb, Wk_sb, kTd, bk_sb, 1.0)

                def attention(h, tj):
                    ft, po = h // HPF, (h % HPF) * HD
                    QhT = QT_sb[po:po + HD, ft, :]
                    KhT = KT_sb[po:po + HD, ft, :]
                    nblk = NR * tj + NR

                    def blk_c0(i):
                        # columns [0, 128r) of diagonal block r are entirely
                        # above the causal boundary -- skip them on all engines
                        r = i - NR * tj
                        return 128 * r if r > 0 else 0

                    def emit_S(i):
                        ps = pS.tile([128, TQ], F32, tag="pS")
                        r = i - NR * tj
                        c0 = blk_c0(i)
                        nc.tensor.matmul(
                            ps[:, c0:],
                            lhsT=KhT[:, i * 128:(i + 1) * 128],
                            rhs=QhT[:, tj * TQ + c0:(tj + 1) * TQ],
                            start=True, stop=(r < 0))
                        if r >= 0:
                            nc.tensor.matmul(
                                ps[:, c0:],
                                lhsT=ident_sb[:],
                                rhs=mneg_sb[:, r, c0:],
                                start=False, stop=True)
                        return ps

                    po_t = pO.tile([HD + 1, TQ], F32, tag="pO")
                    ps_cur = emit_S(0)
                    for i in range(nblk):
                        c0 = blk_c0(i)
                        ps_next = emit_S(i + 1) if i + 1 < nblk else None
                        pt = ptile.tile([128, TQ], F32R, tag="pt")
                        nc.scalar.activation(pt[:, c0:], ps_cur[:, c0:], AF.Exp)
                        nc.tensor.matmul(
                            po_t[:, c0:],
                            lhsT=VA_sb[:, i, h, :],
                            rhs=pt[:, c0:],
                            start=(i == 0), stop=(i == nblk - 1))
                        ps_cur = ps_next
                    den1 = denp.tile([1, TQ], F32, tag="den1")
                    nc.vector.reciprocal(den1[:], po_t[HD:HD + 1, :])
                    dend = dendp.tile([1, TQ], F32, tag="dend")
                    nc.sync.dma_start(dend[:], den1[:])
                    denr = denp.tile([HD, TQ], F32, tag="denr")
                    nc.sync.dma_start(denr[:], dend[0:1, :].to_broadcast([HD, TQ]))
                    nc.vector.tensor_tensor(
                        CTX_sb[po:po + HD, ft, tj * TQ:(tj + 1) * TQ],
                        po_t[0:HD, :], denr[:], OP.mult)

                def qk_proj_ft(which, tj, xt, ft):
                    dst, Wsb, xdram, bsb, sc = which
                    ps = ppool.tile([128, TQ], F32, tag="pproj")
                    for dt in range(DT):
                        nc.tensor.matmul(
                            ps[:],
                            lhsT=Wsb[:, dt, ft * 128:(ft + 1) * 128],
                            rhs=xt[dt][:],
                            start=(dt == 0), stop=(dt == DT - 1))
                    nc.vector.tensor_scalar(
                        dst[:, ft, tj * TQ:(tj + 1) * TQ], ps[:],
                        sc, bsb[:, ft:ft + 1], OP.mult, OP.add)

                # prologue: V for tj=0, then per-feature-tile Q/K proj
                # interleaved with that tile's two heads of attention, so the
                # scalar engine starts exp as early as possible.
                v_proj(0)
                load_w(Wq_sb, Wq)
                nc.sync.dma_start(bq_sb[:], bq[:])
                nc.sync.dma_start(mneg_sb[:], mneg[:])
                nc.sync.dma_start(ident_sb[:], ident[:])
                qk_proj(QSPEC, 0)
                load_w(Wk_sb, Wk)
                nc.sync.dma_start(bk_sb[:], bk[:])
                nc.sync.dma_start(
                    VA_sb[:].rearrange("p a b c -> p (a b) c")[:, :, HD:HD + 1],
                    ones[0:1, 0:1].to_broadcast([128, NTOK * NH, 1]).bitcast(F32R))
                qk_proj(KSPEC, 0)

                for tj in range(NTQ):
                    for h in range(NH):
                        attention(h, tj)
                        if tj + 1 < NTQ:
                            if h == 1:
                                v_proj(tj + 1)
                            elif h == 3:
                                qk_proj(QSPEC, tj + 1)
                            elif h == 5:
                                qk_proj(KSPEC, tj + 1)

            # ---- Phase 3: output projection ----
            with tc.tile_pool(name="wom", bufs=1) as wop, \
                 tc.tile_pool(name="osb", bufs=3) as osb:
                Wo_sb = wop.tile([128, FT, DOUT], F32R)
                nc.sync.dma_start(
                    Wo_sb[:], Wo[:].rearrange("(ft p) n -> p ft n", p=128).bitcast(F32R))
                for tt in range(NTOK):
                    for n in range(NO):
                        ps = ppool.tile([128, NCH], F32, tag="pproj")
                        for ft in range(FT):
                            nc.tensor.matmul(
                                ps[:],
                                lhsT=CTX_sb[:, ft, tt * 128:(tt + 1) * 128],
                                rhs=Wo_sb[:, ft, n * NCH:(n + 1) * NCH],
                                start=(ft == 0), stop=(ft == FT - 1))
                        ot = osb.tile([128, NCH], F32, tag="ot")
                        nc.vector.tensor_copy(ot[:], ps[:])
                        nc.sync.dma_start(
                            out[tt * 128:(tt + 1) * 128, n * NCH:(n + 1) * NCH], ot[:])

    nc.compile()
    return nc


def make_mask(TQ=512, NR=4):
    """mneg[p, r, f] = -1e30 where tk > tq (tk = 128*i+p, tq = tj*TQ+f, r = i-NR*tj)."""
    p = np.arange(128)[:, None, None]
    r = np.arange(NR)[None, :, None]
    f = np.arange(TQ)[None, None, :]
    m = np.where(f < p + 128 * r, np.float32(-1e30), np.float32(0.0))
    return m.astype(ml_dtypes.bfloat16)


def make_core_inputs(q_b, k_b, v_b, Wq, bq, Wk, bk, Wv, bv, Wo, fsl, scale, TQ=512):
    """Build the in_map for one core. fsl = feature slice for this core's heads."""
    F = fsl.stop - fsl.start
    FT = F // 128
    NR = TQ // 128
    return {
        "qT": np.ascontiguousarray(q_b.T),
        "kT": np.ascontiguousarray(k_b.T),
        "vT": np.ascontiguousarray(v_b.T),
        "Wq": np.ascontiguousarray(Wq[:, fsl]),
        "Wk": np.ascontiguousarray(Wk[:, fsl]),
        "Wv": np.ascontiguousarray(Wv[:, fsl]),
        "Wo": np.ascontiguousarray(Wo[fsl, :]),
        "bq": np.ascontiguousarray((bq[fsl] * scale).reshape(FT, 128).T),
        "bk": np.ascontiguousarray(bk[fsl].reshape(FT, 128).T),
        "bv": np.ascontiguousarray(bv[fsl].reshape(1, F)),
        "ones": np.ones((1, 1), np.float32),
        "mneg": make_mask(TQ, NR),
        "ident": np.eye(128, dtype=np.float32).astype(ml_dtypes.bfloat16),
    }


_CACHE = {}


def kernel(q, k, v, Wq, bq, Wk, bk, Wv, bv, Wo, bo, _trace=False):
    B, T, D = q.shape
    H, HD = 16, 64
    scale = np.float32(1.0 / np.sqrt(D))
    n_cores = 8
    gpb = n_cores // B            # head-groups per batch element (2)
    F = D // gpb                  # feature columns per core (512)

    key = (T, D, F)
    if key not in _CACHE:
        _CACHE[key] = build_mha_core(T=T, D=D, F=F, DOUT=D, HD=HD, TQ=512,
                                     scale=float(scale), num_devices=n_cores)
    nc = _CACHE[key]

    q = np.asarray(q, np.float32)
    k = np.asarray(k, np.float32)
    v = np.asarray(v, np.float32)
    in_maps = []
    for c in range(n_cores):
        b, g = c // gpb, c % gpb
        fsl = slice(g * F, (g + 1) * F)
        in_maps.append(make_core_inputs(
            q[b], k[b], v[b], Wq, bq, Wk, bk, Wv, bv, Wo, fsl, scale))

    res = run_bass_kernel_spmd(nc, in_maps, list(range(n_cores)), trace=_trace)
    out = np.zeros((B, T, D), np.float32)
    for c in range(n_cores):
        out[c // gpb] += res.results[c]["out"]
    out += np.asarray(bo, np.float32)
    if _trace:
        kernel.last_exec_time_ns = res.exec_time_ns
    return out
